# revision 20
# baseline (speedup 1.0000x reference)
"""Trainium2 Bass kernel for the DPLSTM problem (2-layer LSTM with adaptive
gate). Data-parallel over batch: 64 rows -> 8 NeuronCores x 8 rows.

Per-core formulation (everything transposed so elementwise work uses all 128
SBUF partitions):
  - Gate order is remapped to [o | f | i | a | g] (each 512 wide, a from W_ag)
    giving a 2560-wide "gate" dim D = 20 m-tiles of 128.
  - Bulk phase: z = x @ W~_ih + (b_ih + b_hh | b_ag) computed with full-array
    matmuls (x^T streamed 512 cols at a time), staged to HBM as bf16.
  - Recurrent phase: per step, G^T tiles [128, 8] = sum_k W~tile[k,m].T @ h^T
    accumulated in PSUM; sigmoid/tanh slabs on ScalarE; cell update on
    VectorE; h written back as bf16 directly into the rhs layout the next
    step's matmuls read.
"""

import os
import numpy as np
import ml_dtypes

B, S, I, H, L = 64, 512, 512, 512, 2
NCORES = 8
BC = B // NCORES          # 8 batch rows per core
KCH = H // 128            # 4 contraction chunks
NT = 20                   # gate m-tiles: [o0-3 f0-3 i0-3 a0-3 g0-3]
NSIG = 16                 # first 16 tiles are sigmoid gates
U = 8                     # steps unrolled per For_i iteration

bf16 = ml_dtypes.bfloat16

_CACHE = {}


def _reorder_cols(W_hh_l, W_ag_l):
    """[H, 4H], [H, H] -> [H, 2560] with column order [o f i a g]."""
    i_g = W_hh_l[:, 0 * H:1 * H]
    f_g = W_hh_l[:, 1 * H:2 * H]
    g_g = W_hh_l[:, 2 * H:3 * H]
    o_g = W_hh_l[:, 3 * H:4 * H]
    return np.concatenate([o_g, f_g, i_g, W_ag_l, g_g], axis=1)


def _tile_w(Wfull):
    """[H, 2560] -> [128, KCH, NT, 128]  (lhsT tiles, K on partitions)."""
    # Wfull[k*128+p, m*128+c] -> out[p, k, m, c]
    Wt = Wfull.reshape(KCH, 128, NT, 128)
    return np.ascontiguousarray(Wt.transpose(1, 0, 2, 3))


def _build_program(s_steps):
    import concourse.bass as bass
    import concourse.tile as tile
    from concourse import bacc, mybir

    dt = mybir.dt
    f32, bf = dt.float32, dt.bfloat16
    AF = mybir.ActivationFunctionType
    ALU = mybir.AluOpType

    nc = bacc.Bacc("TRN2", target_bir_lowering=False, debug=False,
                   num_devices=NCORES)

    xT = nc.dram_tensor("xT", [128, KCH, s_steps, BC], bf, kind="ExternalInput")
    Wr = nc.dram_tensor("Wr", [L, 128, KCH, NT, 128], bf, kind="ExternalInput")
    Wi = nc.dram_tensor("Wi", [L, 128, KCH, NT, 128], bf, kind="ExternalInput")
    Bz = nc.dram_tensor("Bz", [L, 128, NT], f32, kind="ExternalInput")
    outT = nc.dram_tensor("outT", [128, s_steps, KCH, BC], f32,
                          kind="ExternalOutput")
    hnT = nc.dram_tensor("hnT", [128, KCH, BC], f32, kind="ExternalOutput")
    cnT = nc.dram_tensor("cnT", [128, L, KCH, BC], f32, kind="ExternalOutput")
    # z staging in HBM (padded by U steps so in-loop prefetch never runs off)
    zd = [nc.dram_tensor(f"z{l}d", [128, NT, s_steps + U, BC], bf)
          for l in range(L)]

    nsteps = s_steps
    nchunks = nsteps // 64  # bulk N-chunks of 64 steps (512 cols)

    with tile.TileContext(nc) as tc:
        with (
            tc.tile_pool(name="const", bufs=1) as cpool,
            tc.tile_pool(name="warena", bufs=1) as wpool,
            tc.tile_pool(name="bulkps", bufs=2, space="PSUM") as bulk_ps,
            tc.tile_pool(name="bulksb", bufs=4) as bulk_sb,
            tc.tile_pool(name="recps", bufs=2, space="PSUM") as rec_ps,
            tc.tile_pool(name="recsb", bufs=6) as rec_sb,
            tc.tile_pool(name="zpool", bufs=4) as zpool,
            tc.tile_pool(name="opool", bufs=4) as opool,
        ):
            # ---- load constants ----
            w_r = []
            w_i = []
            bias = []
            for l in range(L):
                t = wpool.tile([128, KCH, NT, 128], bf, tag=f"wr{l}")
                nc.gpsimd.dma_start(t[:], Wr[l])
                w_r.append(t)
                t = wpool.tile([128, KCH, NT, 128], bf, tag=f"wi{l}")
                nc.gpsimd.dma_start(t[:], Wi[l])
                w_i.append(t)
                t = cpool.tile([128, NT], f32, tag=f"bz{l}")
                nc.gpsimd.dma_start(t[:], Bz[l])
                bias.append(t)

            # x^T staged in SBUF for the bulk matmuls
            xs = cpool.tile([128, KCH, nsteps, BC], bf, tag="xs")
            nc.gpsimd.dma_start(xs[:], xT[:])

            # h1 arena holds every step's h (slot t+1 = h after step t);
            # slot 0 is the zero initial state. Layer 2 keeps a 2-slot
            # ping-pong (nothing downstream needs its history).
            h1a = cpool.tile([128, nsteps + 1, KCH, BC], bf, tag="h1a")
            h1p = cpool.tile([128, 2, KCH, BC], bf, tag="h1p")
            h2p = cpool.tile([128, 2, KCH, BC], bf, tag="h2p")
            nc.vector.memset(h1a[:, 0:1, :, :], 0.0)
            nc.vector.memset(h1p[:], 0.0)
            nc.vector.memset(h2p[:], 0.0)

            # persistent per-layer cell/aux state
            cg = [cpool.tile([128, 2 * KCH, BC], f32, tag=f"cg{l}", name=f"cg{l}")
                  for l in range(L)]  # [c | g] slab
            for l in range(L):
                nc.vector.memset(cg[l][:], 0.0)
            hf_last = [cpool.tile([128, KCH, BC], f32, tag="hf0", name="hf0")]

            # ---- bulk: z[l] = rhs @ W~_ih[l] + bias -> HBM ----
            def bulk(l, rhs_of, nlo=0, nhi=None):
                # rhs_of(k, c0, cols) -> AP [128, cols steps*BC]
                ngrp = 2  # psum banks used at once
                if nhi is None:
                    nhi = nchunks
                for m in [mm for mm in range(NT) if not 12 <= mm < 16]:
                    for n0 in range(nlo, nhi, ngrp):
                        nn = min(ngrp, nhi - n0)
                        pst = [bulk_ps.tile([128, 512], f32, tag="bps",
                                            name="bps") for _ in range(nn)]
                        for k in range(KCH):
                            lhsT = w_i[l][:, k, m, :]
                            for j in range(nn):
                                nc.tensor.matmul(
                                    pst[j][:], lhsT,
                                    rhs_of(k, (n0 + j) * 64, 64),
                                    start=(k == 0), stop=(k == KCH - 1))
                        for j in range(nn):
                            n = n0 + j
                            ze = bulk_sb.tile([128, 512], bf, tag="zev")
                            nc.vector.tensor_scalar_add(
                                ze[:], pst[j][:], bias[l][:, m:m + 1])
                            nc.sync.dma_start(
                                zd[l][:, m, n * 64:(n + 1) * 64, :],
                                ze[:].rearrange("p (s b) -> p s b", b=BC))

            # ---- one recurrent step ----
            def step(l, t_ap, u, zch, h_rhs_of, h_write, out_slot,
                     lazy_write=None):
                """t_ap: dynamic base step index (t = t_ap), u: static offset.
                zch: SBUF z chunk tile [128, NT, U, BC] for this body.
                h_rhs_of(k) -> AP [128, BC] of h_{t-1}^T chunk k.
                h_write: AP [128, KCH, BC] bf16 destination for h_t.
                out_slot: AP [128, KCH, BC] f32 or None."""
                # three PSUM tiles in distinct banks so downstream reads
                # never touch a bank PE is still writing:
                #   ps_g: g-gate, ps_fia: f/i/a gates, ps_o: o-gate (last)
                ps_g = rec_ps.tile([128, NT - NSIG, BC], f32, tag="psg")
                ps_fia = rec_ps.tile([128, 12, BC], f32, tag="psfia")
                ps_o = rec_ps.tile([128, KCH, BC], f32, tag="pso")
                # m-tile order: g first (tanh path), then f,i,a (cell
                # update), o last (only needed for the final h product)
                for m in (list(range(NSIG, NT)) + list(range(KCH, NSIG))
                          + list(range(KCH))):
                    if m >= NSIG:
                        out = ps_g[:, m - NSIG, :]
                    elif m >= KCH:
                        out = ps_fia[:, m - KCH, :]
                    else:
                        out = ps_o[:, m, :]
                    for k in range(KCH):
                        nc.tensor.matmul(
                            out, w_r[l][:, k, m, :], h_rhs_of(k),
                            start=(k == 0), stop=(k == KCH - 1))
                gz_g = rec_sb.tile([128, NT - NSIG, BC], f32, tag="gzg")
                nc.vector.tensor_add(gz_g[:], ps_g[:], zch[:, NSIG:NT, u, :])
                sgg = rec_sb.tile([128, NT - NSIG, BC], f32, tag="sgg")
                nc.scalar.activation(sgg[:], gz_g[:], AF.Sigmoid, scale=2.0)
                nc.vector.tensor_scalar(
                    cg[l][:, KCH:2 * KCH, :], sgg[:], 2.0, -1.0,
                    ALU.mult, ALU.add)
                gz_s = rec_sb.tile([128, 12, BC], f32, tag="gzs")
                nc.vector.tensor_add(gz_s[:], ps_fia[:],
                                     zch[:, KCH:NSIG, u, :])
                sg = rec_sb.tile([128, 12, BC], f32, tag="sg")
                nc.scalar.activation(sg[:], gz_s[:], AF.Sigmoid)
                # order in sg: [f i a]; cg: [c | g]
                m2 = rec_sb.tile([128, 2 * KCH, BC], f32, tag="m2")
                nc.vector.tensor_mul(m2[:], sg[:, 0:2 * KCH, :], cg[l][:])
                fc = m2[:, 0:KCH, :]
                ig = m2[:, KCH:2 * KCH, :]
                d = rec_sb.tile([128, KCH, BC], f32, tag="d")
                nc.vector.tensor_sub(d[:], fc, ig)
                e = rec_sb.tile([128, KCH, BC], f32, tag="e")
                nc.vector.tensor_mul(e[:], sg[:, 2 * KCH:3 * KCH, :], d[:])
                nc.vector.tensor_add(cg[l][:, 0:KCH, :], e[:], ig)
                tc_ = rec_sb.tile([128, KCH, BC], f32, tag="tc")
                nc.scalar.activation(tc_[:], cg[l][:, 0:KCH, :], AF.Sigmoid,
                                     scale=2.0)
                nc.vector.tensor_scalar(tc_[:], tc_[:], 2.0, -1.0,
                                        ALU.mult, ALU.add)
                gz_o = rec_sb.tile([128, KCH, BC], f32, tag="gzo")
                nc.vector.tensor_add(gz_o[:], ps_o[:], zch[:, 0:KCH, u, :])
                so = rec_sb.tile([128, KCH, BC], f32, tag="so")
                nc.scalar.activation(so[:], gz_o[:], AF.Sigmoid)
                if out_slot is not None:
                    # L2: bf16 product straight into the next matmul's rhs
                    # slot (critical path); an independent fp32 product for
                    # the output runs lazily off-path.
                    nc.vector.tensor_mul(h_write, so[:], tc_[:])
                    nc.vector.tensor_mul(out_slot, so[:], tc_[:])
                else:
                    # L1: write bf16 straight into the next matmul's rhs slot
                    # (removes a serial cast); fp32 copy for hnT is lazy.
                    nc.vector.tensor_mul(h_write, so[:], tc_[:])
                    nc.vector.tensor_copy(hf_last[0][:], h_write)
                if lazy_write is not None:
                    nc.vector.tensor_copy(lazy_write, h_write)

            # a-gate z is just b_ag: fill zd[:, 12:16, :, :] once
            for l in range(L):
                za = bulk_sb.tile([128, 4, 64, BC], bf, tag="zev", name="za")
                nc.vector.memset(za[:], 0.0)
                for j in range(4):
                    nc.vector.tensor_scalar_add(
                        za[:, j, :, :], za[:, j, :, :],
                        bias[l][:, 12 + j:13 + j])
                for s0 in range(0, nsteps + U, 64):
                    sn = min(64, nsteps + U - s0)
                    nc.sync.dma_start(zd[l][:, 12:16, s0:s0 + sn, :],
                                      za[:, :, 0:sn, :])

            # ================= phase 1: bulk z1 from x =================
            bulk(0, lambda k, c0, cols: xs[:, k, c0:c0 + cols, :])

            # ====== pipeline: L2 lags L1 by half the sequence =========
            half = nsteps // 2
            LAG = half

            def l1_prefetch(tb):
                zch = zpool.tile([128, NT, U, BC], bf, tag="z1c", name="z1c")
                nc.sync.dma_start(zch[:], zd[0][:, :, bass.ds(tb, U), :])
                return zch

            def l1_step_u(tb, zch, u):
                step(
                    0, tb, u, zch,
                    h_rhs_of=lambda k: h1p[:, (u % 2), k, :],
                    h_write=h1p[:, ((u + 1) % 2), :, :],
                    out_slot=None,
                    lazy_write=h1a[:, bass.ds(tb + u + 1, 1), :, :])

            def l1_steps(tb):
                zch = l1_prefetch(tb)
                for u in range(U):
                    l1_step_u(tb, zch, u)

            def l2_prefetch(tb, lag):
                zch = zpool.tile([128, NT, U, BC], bf, tag="z2c", name="z2c")
                nc.sync.dma_start(zch[:], zd[1][:, :, bass.ds(tb - lag, U), :])
                och = opool.tile([128, U, KCH, BC], f32, tag="oc", name="oc")
                return zch, och

            def l2_step_u(tb, zch, och, u):
                step(
                    1, tb, u, zch,
                    h_rhs_of=lambda k: h2p[:, (u % 2), k, :],
                    h_write=h2p[:, ((u + 1) % 2), :, :],
                    out_slot=och[:, u, :, :])

            def l2_flush(tb, lag, och):
                nc.gpsimd.dma_start(outT[:, bass.ds(tb - lag, U), :, :],
                                    och[:])

            def l2_steps(tb, lag):
                zch, och = l2_prefetch(tb, lag)
                for u in range(U):
                    l2_step_u(tb, zch, och, u)
                l2_flush(tb, lag, och)

            # phase 2: L1 alone over the first half
            _hint = (mybir.EngineType.PE, mybir.EngineType.DVE,
                     mybir.EngineType.Activation)
            with tc.For_i(0, half, U, name="rec1", staggered_reset=True,
                          hint_engines=_hint) as tb:
                l1_steps(tb)

            # phase 3: z2 for the first half (h1 slots 1..half available)
            bulk(1, lambda k, c0, cols: h1a[:, 1 + c0:1 + c0 + cols, k, :],
                 nlo=0, nhi=half // 64)

            # phase 4: merged loop - L1 second half + L2 first half
            with tc.For_i(half, nsteps, U, name="recm",
                          staggered_reset=True, hint_engines=_hint) as tb:
                zch1 = l1_prefetch(tb)
                zch2, och = l2_prefetch(tb, LAG)
                for u in range(U):
                    l1_step_u(tb, zch1, u)
                    l2_step_u(tb, zch2, och, u)
                l2_flush(tb, LAG, och)

            # phase 5: z2 for the second half
            bulk(1, lambda k, c0, cols: h1a[:, 1 + c0:1 + c0 + cols, k, :],
                 nlo=half // 64, nhi=nchunks)

            # phase 6: L2 alone over the second half
            with tc.For_i(half, nsteps, U, name="rec2",
                          staggered_reset=True, hint_engines=_hint) as tb:
                l2_steps(tb, 0)

            # ================= finals ==================================
            cfin = cpool.tile([128, L, KCH, BC], f32, tag="cfin")
            for l in range(L):
                nc.vector.tensor_copy(cfin[:, l, :, :],
                                      cg[l][:, 0:KCH, :])
            nc.sync.dma_start(hnT[:], hf_last[0][:])
            nc.sync.dma_start(cnT[:], cfin[:])

    nc.compile()
    return nc


def _prep_inputs(x, W_ih, b_ih, W_hh, b_hh, W_ag, b_ag, s_steps):
    """Build per-core input maps (numpy)."""
    Wr_np = np.stack([_tile_w(_reorder_cols(np.asarray(W_hh[l]),
                                            np.asarray(W_ag[l])))
                      for l in range(L)]).astype(bf16)
    Wi_full = []
    for l in range(L):
        ih = np.asarray(W_ih[l])
        i_g = ih[:, 0 * H:1 * H]
        f_g = ih[:, 1 * H:2 * H]
        g_g = ih[:, 2 * H:3 * H]
        o_g = ih[:, 3 * H:4 * H]
        a_g = np.zeros((I, H), np.float32)
        Wi_full.append(_tile_w(np.concatenate([o_g, f_g, i_g, a_g, g_g], 1)))
    Wi_np = np.stack(Wi_full).astype(bf16)

    Bz_np = np.zeros((L, 128, NT), np.float32)
    for l in range(L):
        bb = np.asarray(b_ih[l]) + np.asarray(b_hh[l])
        i_b, f_b, g_b, o_b = (bb[j * H:(j + 1) * H] for j in range(4))
        full = np.concatenate([o_b, f_b, i_b, np.asarray(b_ag[l]), g_b])
        Bz_np[l] = full.reshape(NT, 128).T

    in_maps = []
    xx = np.asarray(x)[:, :s_steps, :]
    for c in range(NCORES):
        xc = xx[c * BC:(c + 1) * BC]            # [BC, s, I]
        # xT[p, k, s, b] = x[b, s, k*128+p]
        xt = xc.transpose(2, 1, 0).reshape(KCH, 128, s_steps, BC)
        xt = np.ascontiguousarray(xt.transpose(1, 0, 2, 3)).astype(bf16)
        in_maps.append({"xT": xt, "Wr": Wr_np, "Wi": Wi_np, "Bz": Bz_np})
    return in_maps


def _assemble(results, s_steps):
    out = np.empty((B, s_steps, H), np.float32)
    h_n = np.empty((1, L, B, H), np.float32)
    c_n = np.empty((1, L, B, H), np.float32)
    for c, r in enumerate(results):
        # outT [128, s, KCH, BC] -> out[b, s, kch*128+p]
        o = r["outT"]
        out[c * BC:(c + 1) * BC] = o.transpose(3, 1, 2, 0).reshape(
            BC, s_steps, H)
        hn = r["hnT"]  # [128, KCH, BC] (layer 1 only)
        cn = r["cnT"]
        h_n[0, 0, c * BC:(c + 1) * BC, :] = hn.transpose(2, 1, 0).reshape(BC, H)
        h_n[0, 1, c * BC:(c + 1) * BC, :] = out[c * BC:(c + 1) * BC, -1, :]
        c_n[0, :, c * BC:(c + 1) * BC, :] = cn.transpose(1, 3, 2, 0).reshape(
            L, BC, H)
    return out, (h_n, c_n)


def _install_ntff_shim():
    """Provide antenv.axon_hooks (missing in this image) so trace=True can
    capture NTFF profiles. Returns True if the hook is usable."""
    import sys
    import types
    try:
        from antenv.axon_hooks import get_axon_ntff_profile_hook  # noqa: F401
        return True
    except ImportError:
        pass
    try:
        from trn_agent_boot.trn_boot import _ntff_profile_via_ctypes
        hook = _ntff_profile_via_ctypes("/opt/axon/libaxon_pjrt.so")
        if hook is None:
            return False
        mod = types.ModuleType("antenv.axon_hooks")
        mod._hook = hook
        mod.set_axon_ntff_profile_hook = lambda h: setattr(mod, "_hook", h)
        mod.get_axon_ntff_profile_hook = lambda: mod._hook
        sys.modules["antenv.axon_hooks"] = mod
        return True
    except Exception:
        return False


def kernel(x, W_ih, b_ih, W_hh, b_hh, W_ag, b_ag, _s_steps=None):
    from concourse.bass_utils import run_bass_kernel_spmd

    s_steps = _s_steps or S
    if s_steps not in _CACHE:
        _CACHE[s_steps] = _build_program(s_steps)
    nc = _CACHE[s_steps]
    in_maps = _prep_inputs(x, W_ih, b_ih, W_hh, b_hh, W_ag, b_ag, s_steps)
    trace = bool(int(os.environ.get("KERNEL_TRACE", "0")))
    if trace:
        trace = _install_ntff_shim()
    res = run_bass_kernel_spmd(nc, in_maps, core_ids=list(range(NCORES)),
                               trace=trace)
    if trace and res.exec_time_ns is not None:
        print(f"HW exec time: {res.exec_time_ns} ns")
        kernel.last_exec_time_ns = res.exec_time_ns
    return _assemble(res.results, s_steps)


# revision 21
# speedup vs baseline: 1.0018x; 1.0018x over previous
"""Trainium2 Bass kernel for the DPLSTM problem (2-layer LSTM with adaptive
gate). Data-parallel over batch: 64 rows -> 8 NeuronCores x 8 rows.

Per-core formulation (everything transposed so elementwise work uses all 128
SBUF partitions):
  - Gate order is remapped to [o | f | i | a | g] (each 512 wide, a from W_ag)
    giving a 2560-wide "gate" dim D = 20 m-tiles of 128.
  - Bulk phase: z = x @ W~_ih + (b_ih + b_hh | b_ag) computed with full-array
    matmuls (x^T streamed 512 cols at a time), staged to HBM as bf16.
  - Recurrent phase: per step, G^T tiles [128, 8] = sum_k W~tile[k,m].T @ h^T
    accumulated in PSUM; sigmoid/tanh slabs on ScalarE; cell update on
    VectorE; h written back as bf16 directly into the rhs layout the next
    step's matmuls read.
"""

import os
import numpy as np
import ml_dtypes

B, S, I, H, L = 64, 512, 512, 512, 2
NCORES = 8
BC = B // NCORES          # 8 batch rows per core
KCH = H // 128            # 4 contraction chunks
NT = 20                   # gate m-tiles: [o0-3 f0-3 i0-3 a0-3 g0-3]
NSIG = 16                 # first 16 tiles are sigmoid gates
U = 8                     # steps unrolled per For_i iteration

bf16 = ml_dtypes.bfloat16

_CACHE = {}


def _reorder_cols(W_hh_l, W_ag_l):
    """[H, 4H], [H, H] -> [H, 2560] with column order [o f i a g]."""
    i_g = W_hh_l[:, 0 * H:1 * H]
    f_g = W_hh_l[:, 1 * H:2 * H]
    g_g = W_hh_l[:, 2 * H:3 * H]
    o_g = W_hh_l[:, 3 * H:4 * H]
    return np.concatenate([o_g, f_g, i_g, W_ag_l, g_g], axis=1)


def _tile_w(Wfull):
    """[H, 2560] -> [128, KCH, NT, 128]  (lhsT tiles, K on partitions)."""
    # Wfull[k*128+p, m*128+c] -> out[p, k, m, c]
    Wt = Wfull.reshape(KCH, 128, NT, 128)
    return np.ascontiguousarray(Wt.transpose(1, 0, 2, 3))


def _build_program(s_steps):
    import concourse.bass as bass
    import concourse.tile as tile
    from concourse import bacc, mybir

    dt = mybir.dt
    f32, bf = dt.float32, dt.bfloat16
    AF = mybir.ActivationFunctionType
    ALU = mybir.AluOpType

    nc = bacc.Bacc("TRN2", target_bir_lowering=False, debug=False,
                   num_devices=NCORES)

    xT = nc.dram_tensor("xT", [128, KCH, s_steps, BC], bf, kind="ExternalInput")
    Wr = nc.dram_tensor("Wr", [L, 128, KCH, NT, 128], bf, kind="ExternalInput")
    Wi = nc.dram_tensor("Wi", [L, 128, KCH, NT, 128], bf, kind="ExternalInput")
    Bz = nc.dram_tensor("Bz", [L, 128, NT], f32, kind="ExternalInput")
    outT = nc.dram_tensor("outT", [128, s_steps, KCH, BC], f32,
                          kind="ExternalOutput")
    hnT = nc.dram_tensor("hnT", [128, KCH, BC], f32, kind="ExternalOutput")
    cnT = nc.dram_tensor("cnT", [128, L, KCH, BC], f32, kind="ExternalOutput")
    # z staging in HBM (padded by U steps so in-loop prefetch never runs off)
    zd = [nc.dram_tensor(f"z{l}d", [128, NT, s_steps + U, BC], bf)
          for l in range(L)]

    nsteps = s_steps
    nchunks = nsteps // 64  # bulk N-chunks of 64 steps (512 cols)

    with tile.TileContext(nc) as tc:
        with (
            tc.tile_pool(name="const", bufs=1) as cpool,
            tc.tile_pool(name="warena", bufs=1) as wpool,
            tc.tile_pool(name="bulkps", bufs=2, space="PSUM") as bulk_ps,
            tc.tile_pool(name="bulksb", bufs=4) as bulk_sb,
            tc.tile_pool(name="recps", bufs=2, space="PSUM") as rec_ps,
            tc.tile_pool(name="recsb", bufs=6) as rec_sb,
            tc.tile_pool(name="zpool", bufs=4) as zpool,
            tc.tile_pool(name="opool", bufs=4) as opool,
        ):
            # ---- load constants ----
            w_r = []
            w_i = []
            bias = []
            for l in range(L):
                t = wpool.tile([128, KCH, NT, 128], bf, tag=f"wr{l}")
                nc.gpsimd.dma_start(t[:], Wr[l])
                w_r.append(t)
                t = wpool.tile([128, KCH, NT, 128], bf, tag=f"wi{l}")
                nc.gpsimd.dma_start(t[:], Wi[l])
                w_i.append(t)
                t = cpool.tile([128, NT], f32, tag=f"bz{l}")
                nc.gpsimd.dma_start(t[:], Bz[l])
                bias.append(t)

            # x^T staged in SBUF for the bulk matmuls
            xs = cpool.tile([128, KCH, nsteps, BC], bf, tag="xs")
            nc.gpsimd.dma_start(xs[:], xT[:])

            # h1 arena holds every step's h (slot t+1 = h after step t);
            # slot 0 is the zero initial state. Layer 2 keeps a 2-slot
            # ping-pong (nothing downstream needs its history).
            h1a = cpool.tile([128, nsteps + 1, KCH, BC], bf, tag="h1a")
            h1p = cpool.tile([128, 2, KCH, BC], bf, tag="h1p")
            h2p = cpool.tile([128, 2, KCH, BC], bf, tag="h2p")
            nc.vector.memset(h1a[:, 0:1, :, :], 0.0)
            nc.vector.memset(h1p[:], 0.0)
            nc.vector.memset(h2p[:], 0.0)

            # persistent per-layer cell/aux state
            cg = [cpool.tile([128, 2 * KCH, BC], f32, tag=f"cg{l}", name=f"cg{l}")
                  for l in range(L)]  # [c | g] slab
            for l in range(L):
                nc.vector.memset(cg[l][:], 0.0)
            hf_last = [cpool.tile([128, KCH, BC], f32, tag="hf0", name="hf0")]

            # ---- bulk: z[l] = rhs @ W~_ih[l] + bias -> HBM ----
            def bulk(l, rhs_of, nlo=0, nhi=None):
                # rhs_of(k, c0, cols) -> AP [128, cols steps*BC]
                ngrp = 2  # psum banks used at once
                if nhi is None:
                    nhi = nchunks
                for m in [mm for mm in range(NT) if not 12 <= mm < 16]:
                    for n0 in range(nlo, nhi, ngrp):
                        nn = min(ngrp, nhi - n0)
                        pst = [bulk_ps.tile([128, 512], f32, tag="bps",
                                            name="bps") for _ in range(nn)]
                        for k in range(KCH):
                            lhsT = w_i[l][:, k, m, :]
                            for j in range(nn):
                                nc.tensor.matmul(
                                    pst[j][:], lhsT,
                                    rhs_of(k, (n0 + j) * 64, 64),
                                    start=(k == 0), stop=(k == KCH - 1))
                        for j in range(nn):
                            n = n0 + j
                            ze = bulk_sb.tile([128, 512], bf, tag="zev")
                            nc.vector.tensor_scalar_add(
                                ze[:], pst[j][:], bias[l][:, m:m + 1])
                            nc.sync.dma_start(
                                zd[l][:, m, n * 64:(n + 1) * 64, :],
                                ze[:].rearrange("p (s b) -> p s b", b=BC))

            # ---- one recurrent step ----
            def step(l, t_ap, u, zch, h_rhs_of, h_write, out_slot,
                     lazy_write=None):
                """t_ap: dynamic base step index (t = t_ap), u: static offset.
                zch: SBUF z chunk tile [128, NT, U, BC] for this body.
                h_rhs_of(k) -> AP [128, BC] of h_{t-1}^T chunk k.
                h_write: AP [128, KCH, BC] bf16 destination for h_t.
                out_slot: AP [128, KCH, BC] f32 or None."""
                # three PSUM tiles in distinct banks so downstream reads
                # never touch a bank PE is still writing:
                #   ps_g: g-gate, ps_fia: f/i/a gates, ps_o: o-gate (last)
                ps_g = rec_ps.tile([128, NT - NSIG, BC], f32, tag="psg")
                ps_fia = rec_ps.tile([128, 12, BC], f32, tag="psfia")
                ps_o = rec_ps.tile([128, KCH, BC], f32, tag="pso")
                # m-tile order: g first (tanh path), then f,i,a (cell
                # update), o last (only needed for the final h product)
                for m in (list(range(NSIG, NT)) + list(range(KCH, NSIG))
                          + list(range(KCH))):
                    if m >= NSIG:
                        out = ps_g[:, m - NSIG, :]
                    elif m >= KCH:
                        out = ps_fia[:, m - KCH, :]
                    else:
                        out = ps_o[:, m, :]
                    for k in range(KCH):
                        nc.tensor.matmul(
                            out, w_r[l][:, k, m, :], h_rhs_of(k),
                            start=(k == 0), stop=(k == KCH - 1))
                gz_g = rec_sb.tile([128, NT - NSIG, BC], f32, tag="gzg")
                nc.vector.tensor_add(gz_g[:], ps_g[:], zch[:, NSIG:NT, u, :])
                sgg = rec_sb.tile([128, NT - NSIG, BC], f32, tag="sgg")
                nc.scalar.activation(sgg[:], gz_g[:], AF.Sigmoid, scale=2.0)
                nc.vector.tensor_scalar(
                    cg[l][:, KCH:2 * KCH, :], sgg[:], 2.0, -1.0,
                    ALU.mult, ALU.add)
                gz_s = rec_sb.tile([128, 12, BC], f32, tag="gzs")
                nc.vector.tensor_add(gz_s[:], ps_fia[:],
                                     zch[:, KCH:NSIG, u, :])
                sg = rec_sb.tile([128, 12, BC], f32, tag="sg")
                nc.scalar.activation(sg[:], gz_s[:], AF.Sigmoid)
                # order in sg: [f i a]; cg: [c | g]
                m2 = rec_sb.tile([128, 2 * KCH, BC], f32, tag="m2")
                nc.vector.tensor_mul(m2[:], sg[:, 0:2 * KCH, :], cg[l][:])
                fc = m2[:, 0:KCH, :]
                ig = m2[:, KCH:2 * KCH, :]
                d = rec_sb.tile([128, KCH, BC], f32, tag="d")
                nc.vector.tensor_sub(d[:], fc, ig)
                e = rec_sb.tile([128, KCH, BC], f32, tag="e")
                nc.vector.tensor_mul(e[:], sg[:, 2 * KCH:3 * KCH, :], d[:])
                nc.vector.tensor_add(cg[l][:, 0:KCH, :], e[:], ig)
                tc_ = rec_sb.tile([128, KCH, BC], f32, tag="tc")
                nc.scalar.activation(tc_[:], cg[l][:, 0:KCH, :], AF.Sigmoid,
                                     scale=2.0)
                nc.vector.tensor_scalar(tc_[:], tc_[:], 2.0, -1.0,
                                        ALU.mult, ALU.add)
                gz_o = rec_sb.tile([128, KCH, BC], f32, tag="gzo")
                nc.vector.tensor_add(gz_o[:], ps_o[:], zch[:, 0:KCH, u, :])
                so = rec_sb.tile([128, KCH, BC], f32, tag="so")
                nc.scalar.activation(so[:], gz_o[:], AF.Sigmoid)
                if out_slot is not None:
                    # L2: product must stay fp32 (it IS the output); cast to
                    # the matmul slot afterwards.
                    nc.vector.tensor_mul(out_slot, so[:], tc_[:])
                    nc.vector.tensor_copy(h_write, out_slot)
                else:
                    # L1: write bf16 straight into the next matmul's rhs slot
                    # (removes a serial cast); fp32 copy for hnT is lazy.
                    nc.vector.tensor_mul(h_write, so[:], tc_[:])
                    nc.vector.tensor_copy(hf_last[0][:], h_write)
                if lazy_write is not None:
                    nc.vector.tensor_copy(lazy_write, h_write)

            # a-gate z is just b_ag: fill zd[:, 12:16, :, :] once
            for l in range(L):
                za = bulk_sb.tile([128, 4, 64, BC], bf, tag="zev", name="za")
                nc.vector.memset(za[:], 0.0)
                for j in range(4):
                    nc.vector.tensor_scalar_add(
                        za[:, j, :, :], za[:, j, :, :],
                        bias[l][:, 12 + j:13 + j])
                for s0 in range(0, nsteps + U, 64):
                    sn = min(64, nsteps + U - s0)
                    nc.sync.dma_start(zd[l][:, 12:16, s0:s0 + sn, :],
                                      za[:, :, 0:sn, :])

            # ================= phase 1: bulk z1 from x =================
            bulk(0, lambda k, c0, cols: xs[:, k, c0:c0 + cols, :])

            # ====== pipeline: L2 lags L1 by half the sequence =========
            half = nsteps // 2
            LAG = half

            def l1_prefetch(tb):
                zch = zpool.tile([128, NT, U, BC], bf, tag="z1c", name="z1c")
                nc.sync.dma_start(zch[:], zd[0][:, :, bass.ds(tb, U), :])
                return zch

            def l1_step_u(tb, zch, u):
                step(
                    0, tb, u, zch,
                    h_rhs_of=lambda k: h1p[:, (u % 2), k, :],
                    h_write=h1p[:, ((u + 1) % 2), :, :],
                    out_slot=None,
                    lazy_write=h1a[:, bass.ds(tb + u + 1, 1), :, :])

            def l1_steps(tb):
                zch = l1_prefetch(tb)
                for u in range(U):
                    l1_step_u(tb, zch, u)

            def l2_prefetch(tb, lag):
                zch = zpool.tile([128, NT, U, BC], bf, tag="z2c", name="z2c")
                nc.sync.dma_start(zch[:], zd[1][:, :, bass.ds(tb - lag, U), :])
                och = opool.tile([128, U, KCH, BC], f32, tag="oc", name="oc")
                return zch, och

            def l2_step_u(tb, zch, och, u):
                step(
                    1, tb, u, zch,
                    h_rhs_of=lambda k: h2p[:, (u % 2), k, :],
                    h_write=h2p[:, ((u + 1) % 2), :, :],
                    out_slot=och[:, u, :, :])

            def l2_flush(tb, lag, och):
                nc.gpsimd.dma_start(outT[:, bass.ds(tb - lag, U), :, :],
                                    och[:])

            def l2_steps(tb, lag):
                zch, och = l2_prefetch(tb, lag)
                for u in range(U):
                    l2_step_u(tb, zch, och, u)
                l2_flush(tb, lag, och)

            # phase 2: L1 alone over the first half
            _hint = (mybir.EngineType.PE, mybir.EngineType.DVE,
                     mybir.EngineType.Activation)
            with tc.For_i(0, half, U, name="rec1", staggered_reset=True,
                          hint_engines=_hint) as tb:
                l1_steps(tb)

            # phase 3: z2 for the first half (h1 slots 1..half available)
            bulk(1, lambda k, c0, cols: h1a[:, 1 + c0:1 + c0 + cols, k, :],
                 nlo=0, nhi=half // 64)

            # phase 4: merged loop - L1 second half + L2 first half
            with tc.For_i(half, nsteps, U, name="recm",
                          staggered_reset=True, hint_engines=_hint) as tb:
                zch1 = l1_prefetch(tb)
                zch2, och = l2_prefetch(tb, LAG)
                for u in range(U):
                    l1_step_u(tb, zch1, u)
                    l2_step_u(tb, zch2, och, u)
                l2_flush(tb, LAG, och)

            # phase 5: z2 for the second half
            bulk(1, lambda k, c0, cols: h1a[:, 1 + c0:1 + c0 + cols, k, :],
                 nlo=half // 64, nhi=nchunks)

            # phase 6: L2 alone over the second half
            with tc.For_i(half, nsteps, U, name="rec2",
                          staggered_reset=True, hint_engines=_hint) as tb:
                l2_steps(tb, 0)

            # ================= finals ==================================
            cfin = cpool.tile([128, L, KCH, BC], f32, tag="cfin")
            for l in range(L):
                nc.vector.tensor_copy(cfin[:, l, :, :],
                                      cg[l][:, 0:KCH, :])
            nc.sync.dma_start(hnT[:], hf_last[0][:])
            nc.sync.dma_start(cnT[:], cfin[:])

    nc.compile()
    return nc


def _prep_inputs(x, W_ih, b_ih, W_hh, b_hh, W_ag, b_ag, s_steps):
    """Build per-core input maps (numpy)."""
    Wr_np = np.stack([_tile_w(_reorder_cols(np.asarray(W_hh[l]),
                                            np.asarray(W_ag[l])))
                      for l in range(L)]).astype(bf16)
    Wi_full = []
    for l in range(L):
        ih = np.asarray(W_ih[l])
        i_g = ih[:, 0 * H:1 * H]
        f_g = ih[:, 1 * H:2 * H]
        g_g = ih[:, 2 * H:3 * H]
        o_g = ih[:, 3 * H:4 * H]
        a_g = np.zeros((I, H), np.float32)
        Wi_full.append(_tile_w(np.concatenate([o_g, f_g, i_g, a_g, g_g], 1)))
    Wi_np = np.stack(Wi_full).astype(bf16)

    Bz_np = np.zeros((L, 128, NT), np.float32)
    for l in range(L):
        bb = np.asarray(b_ih[l]) + np.asarray(b_hh[l])
        i_b, f_b, g_b, o_b = (bb[j * H:(j + 1) * H] for j in range(4))
        full = np.concatenate([o_b, f_b, i_b, np.asarray(b_ag[l]), g_b])
        Bz_np[l] = full.reshape(NT, 128).T

    in_maps = []
    xx = np.asarray(x)[:, :s_steps, :]
    for c in range(NCORES):
        xc = xx[c * BC:(c + 1) * BC]            # [BC, s, I]
        # xT[p, k, s, b] = x[b, s, k*128+p]
        xt = xc.transpose(2, 1, 0).reshape(KCH, 128, s_steps, BC)
        xt = np.ascontiguousarray(xt.transpose(1, 0, 2, 3)).astype(bf16)
        in_maps.append({"xT": xt, "Wr": Wr_np, "Wi": Wi_np, "Bz": Bz_np})
    return in_maps


def _assemble(results, s_steps):
    out = np.empty((B, s_steps, H), np.float32)
    h_n = np.empty((1, L, B, H), np.float32)
    c_n = np.empty((1, L, B, H), np.float32)
    for c, r in enumerate(results):
        # outT [128, s, KCH, BC] -> out[b, s, kch*128+p]
        o = r["outT"]
        out[c * BC:(c + 1) * BC] = o.transpose(3, 1, 2, 0).reshape(
            BC, s_steps, H)
        hn = r["hnT"]  # [128, KCH, BC] (layer 1 only)
        cn = r["cnT"]
        h_n[0, 0, c * BC:(c + 1) * BC, :] = hn.transpose(2, 1, 0).reshape(BC, H)
        h_n[0, 1, c * BC:(c + 1) * BC, :] = out[c * BC:(c + 1) * BC, -1, :]
        c_n[0, :, c * BC:(c + 1) * BC, :] = cn.transpose(1, 3, 2, 0).reshape(
            L, BC, H)
    return out, (h_n, c_n)


def _install_ntff_shim():
    """Provide antenv.axon_hooks (missing in this image) so trace=True can
    capture NTFF profiles. Returns True if the hook is usable."""
    import sys
    import types
    try:
        from antenv.axon_hooks import get_axon_ntff_profile_hook  # noqa: F401
        return True
    except ImportError:
        pass
    try:
        from trn_agent_boot.trn_boot import _ntff_profile_via_ctypes
        hook = _ntff_profile_via_ctypes("/opt/axon/libaxon_pjrt.so")
        if hook is None:
            return False
        mod = types.ModuleType("antenv.axon_hooks")
        mod._hook = hook
        mod.set_axon_ntff_profile_hook = lambda h: setattr(mod, "_hook", h)
        mod.get_axon_ntff_profile_hook = lambda: mod._hook
        sys.modules["antenv.axon_hooks"] = mod
        return True
    except Exception:
        return False


def kernel(x, W_ih, b_ih, W_hh, b_hh, W_ag, b_ag, _s_steps=None):
    from concourse.bass_utils import run_bass_kernel_spmd

    s_steps = _s_steps or S
    if s_steps not in _CACHE:
        _CACHE[s_steps] = _build_program(s_steps)
    nc = _CACHE[s_steps]
    in_maps = _prep_inputs(x, W_ih, b_ih, W_hh, b_hh, W_ag, b_ag, s_steps)
    trace = bool(int(os.environ.get("KERNEL_TRACE", "0")))
    if trace:
        trace = _install_ntff_shim()
    res = run_bass_kernel_spmd(nc, in_maps, core_ids=list(range(NCORES)),
                               trace=trace)
    if trace and res.exec_time_ns is not None:
        print(f"HW exec time: {res.exec_time_ns} ns")
        kernel.last_exec_time_ns = res.exec_time_ns
    return _assemble(res.results, s_steps)
